# revision 4
# baseline (speedup 1.0000x reference)
"""CTPN loss kernel for Trainium2 (8 NeuronCores, Bass/Tile).

Strategy
--------
The loss only touches 64 pos + 64 neg anchor locations of the dense
[1,2K,H,W] maps, so the kernel is a sparse gather + tiny reduction.

- Dense tensors are sharded by image rows (H) across the 8 cores:
  core c holds scores/vcoords/sides rows [64c, 64c+64).
- Each anchor is routed to the core that owns its row (y // 64). Per
  core, anchor metadata (flat gather offsets, targets, weights) is
  packed on host into small [64, k] tensors, padded with zero-weight
  rows. The device performs the actual gathers from its HBM shard with
  indirect DMA, computes softplus / smooth-L1 terms, and reduces to 6
  partial sums. The 8 partial-sum vectors are combined on host
  (all-reduce of the scalar losses) and normalized into the 4 outputs.

The compiled program is input-independent (offsets travel as data), so
it is built and compiled once and reused across calls.
"""

import os

import numpy as np

H, W, K = 512, 1024, 10
N_POS = 64
N_NEG = 64
N_CORES = 8
HSH = H // N_CORES          # 64 rows of the image per core
PLANE = HSH * W             # elements per channel per shard
SC_N = 2 * K * PLANE        # scores/vcoords shard flat length
SD_N = K * PLANE            # sides shard flat length
CAP = 64                    # per-core anchor capacity (worst case)

_CACHE = {}
last_exec_time_ns = None


def _build_nc():
    import concourse.bass as bass
    import concourse.tile as tile
    from concourse import bacc, mybir
    from contextlib import ExitStack

    f32 = mybir.dt.float32
    i32 = mybir.dt.int32
    AF = mybir.ActivationFunctionType
    ALU = mybir.AluOpType

    nc = bacc.Bacc(
        "TRN2", target_bir_lowering=False, debug=False, num_devices=N_CORES
    )

    sc = nc.dram_tensor("scores_sh", [SC_N, 1], f32, kind="ExternalInput")
    vc = nc.dram_tensor("vcoords_sh", [SC_N, 1], f32, kind="ExternalInput")
    sd = nc.dram_tensor("sides_sh", [SD_N, 1], f32, kind="ExternalInput")
    # idx columns: 0 s0p, 1 s1n, 2 v0p, 3 v1p, 4 side, 5 s1p, 6 s0n
    idx = nc.dram_tensor("idx", [CAP, 7], i32, kind="ExternalInput")
    # hostdat columns: 0 vt0, 1 vt1, 2 side_t, 3..8 weights for the 6 terms
    hd = nc.dram_tensor("hostdat", [CAP, 9], f32, kind="ExternalInput")
    out = nc.dram_tensor("partials", [1, 6], f32, kind="ExternalOutput")

    with ExitStack() as ctx:
        tc = ctx.enter_context(tile.TileContext(nc))
        pool = ctx.enter_context(tc.tile_pool(name="p", bufs=1))

        idx_t = pool.tile([CAP, 7], i32)
        nc.sync.dma_start(idx_t[:], idx.ap())

        # A: gathered minuends [s0p, s1n, v0, v1, sp]
        # M: subtrahends [s1p, s0n, vt0, vt1, side_t]
        A = pool.tile([CAP, 5], f32)
        M = pool.tile([CAP, 5], f32)
        Wt = pool.tile([CAP, 6], f32)
        nc.sync.dma_start(M[:, 2:5], hd.ap()[:, 0:3])
        nc.sync.dma_start(Wt[:], hd.ap()[:, 3:9])

        gathers = [
            (A, 0, sc, 0),
            (A, 1, sc, 1),
            (A, 2, vc, 2),
            (A, 3, vc, 3),
            (A, 4, sd, 4),
            (M, 0, sc, 5),
            (M, 1, sc, 6),
        ]
        for dst, dcol, src, icol in gathers:
            nc.gpsimd.indirect_dma_start(
                out=dst[:, dcol : dcol + 1],
                out_offset=None,
                in_=src.ap(),
                in_offset=bass.IndirectOffsetOnAxis(
                    ap=idx_t[:, icol : icol + 1], axis=0
                ),
            )

        D = pool.tile([CAP, 5], f32)
        nc.vector.tensor_tensor(out=D[:], in0=A[:], in1=M[:], op=ALU.subtract)

        RES = pool.tile([CAP, 6], f32)
        # cross-entropy terms: -log_softmax picks reduce to softplus(diff);
        # no HW softplus table, so ln(1 + exp(d)) (exp/ln/abs/square share
        # the natural_log_exp_and_others ACT table -> single table load)
        E = pool.tile([CAP, 2], f32)
        nc.scalar.activation(E[:], D[:, 0:2], AF.Exp)
        E1 = pool.tile([CAP, 2], f32)
        nc.vector.tensor_scalar_add(E1[:], E[:], 1.0)
        nc.scalar.activation(RES[:, 0:2], E1[:], AF.Ln)
        # smooth-L1 via sl1(d) = 0.5*min(|d|,1)^2 + |d| - min(|d|,1)
        Aab = pool.tile([CAP, 3], f32)
        nc.scalar.activation(Aab[:], D[:, 2:5], AF.Abs)
        Mn = pool.tile([CAP, 3], f32)
        nc.vector.tensor_scalar_min(Mn[:], Aab[:], 1.0)
        T = pool.tile([CAP, 3], f32)
        nc.vector.tensor_tensor(out=T[:], in0=Aab[:], in1=Mn[:], op=ALU.subtract)
        Q = pool.tile([CAP, 3], f32)
        # Square(scale*x) with scale=sqrt(0.5) gives 0.5*x^2
        nc.scalar.activation(Q[:], Mn[:], AF.Square, scale=0.7071067811865476)
        nc.vector.tensor_tensor(out=RES[:, 2:5], in0=Q[:], in1=T[:], op=ALU.add)
        nc.vector.memset(RES[:, 5:6], 1.0)

        TW = pool.tile([CAP, 6], f32)
        nc.vector.tensor_tensor(out=TW[:], in0=RES[:], in1=Wt[:], op=ALU.mult)

        from concourse import bass_isa

        S = pool.tile([CAP, 6], f32)
        nc.gpsimd.partition_all_reduce(
            S[:], TW[:], channels=CAP, reduce_op=bass_isa.ReduceOp.add
        )
        nc.sync.dma_start(out.ap(), S[0:1, :])

    nc.compile()
    return nc


def _get_nc():
    if "nc" not in _CACHE:
        _CACHE["nc"] = _build_nc()
    return _CACHE["nc"]


def _pack_core_inputs(
    scores, vcoords, sides, pos_y, pos_x, pos_z, neg_y, neg_x, neg_z,
    v_targets, side_mask, side_targets,
):
    """Build the 8 per-core input maps (shards + packed anchor metadata)."""
    scores = np.asarray(scores, dtype=np.float32).reshape(2 * K, H, W)
    vcoords = np.asarray(vcoords, dtype=np.float32).reshape(2 * K, H, W)
    sides = np.asarray(sides, dtype=np.float32).reshape(K, H, W)
    pos_y = np.asarray(pos_y).astype(np.int64)
    pos_x = np.asarray(pos_x).astype(np.int64)
    pos_z = np.asarray(pos_z).astype(np.int64)
    neg_y = np.asarray(neg_y).astype(np.int64)
    neg_x = np.asarray(neg_x).astype(np.int64)
    neg_z = np.asarray(neg_z).astype(np.int64)
    v_targets = np.asarray(v_targets, dtype=np.float32)
    side_mask_f = np.asarray(side_mask).astype(np.float32)
    side_targets = np.asarray(side_targets, dtype=np.float32)

    pc = pos_y // HSH
    nc_ = neg_y // HSH
    pbase = (pos_y % HSH) * W + pos_x
    nbase = (neg_y % HSH) * W + neg_x
    p_s0 = (2 * pos_z) * PLANE + pbase
    p_s1 = (2 * pos_z + 1) * PLANE + pbase
    p_sd = pos_z * PLANE + pbase
    n_s0 = (2 * neg_z) * PLANE + nbase
    n_s1 = (2 * neg_z + 1) * PLANE + nbase

    in_maps = []
    for c in range(N_CORES):
        r = slice(c * HSH, (c + 1) * HSH)
        sel_p = np.nonzero(pc == c)[0]
        sel_n = np.nonzero(nc_ == c)[0]
        npc, nnc = len(sel_p), len(sel_n)

        idx = np.zeros((CAP, 7), dtype=np.int32)
        idx[:npc, 0] = p_s0[sel_p]
        idx[:nnc, 1] = n_s1[sel_n]
        idx[:npc, 2] = p_s0[sel_p]
        idx[:npc, 3] = p_s1[sel_p]
        idx[:npc, 4] = p_sd[sel_p]
        idx[:npc, 5] = p_s1[sel_p]
        idx[:nnc, 6] = n_s0[sel_n]

        hd = np.zeros((CAP, 9), dtype=np.float32)
        hd[:npc, 0] = v_targets[sel_p, 0]
        hd[:npc, 1] = v_targets[sel_p, 1]
        hd[:npc, 2] = side_targets[sel_p]
        hd[:npc, 3] = 1.0                      # cls pos weight
        hd[:nnc, 4] = 1.0                      # cls neg weight
        hd[:npc, 5] = 1.0                      # v0 weight
        hd[:npc, 6] = 1.0                      # v1 weight
        hd[:npc, 7] = side_mask_f[sel_p]       # side term weight
        hd[:npc, 8] = side_mask_f[sel_p]       # side count

        in_maps.append(
            {
                "scores_sh": np.ascontiguousarray(scores[:, r, :]).reshape(SC_N, 1),
                "vcoords_sh": np.ascontiguousarray(vcoords[:, r, :]).reshape(SC_N, 1),
                "sides_sh": np.ascontiguousarray(sides[:, r, :]).reshape(SD_N, 1),
                "idx": idx,
                "hostdat": hd,
            }
        )
    return in_maps


def _install_ntff_hook():
    """The agent image's antenv lacks axon_hooks; synthesize it and wire the
    ctypes NTFF profiling hook from trn_boot so trace=True works."""
    import sys
    import types

    if "antenv.axon_hooks" in sys.modules:
        return True
    try:
        from trn_agent_boot.trn_boot import _ntff_profile_via_ctypes

        hook = _ntff_profile_via_ctypes("/opt/axon/libaxon_pjrt.so")
        if hook is None:
            return False
        mod = types.ModuleType("antenv.axon_hooks")
        mod._hook = hook
        mod.get_axon_ntff_profile_hook = lambda: mod._hook

        def _set(h):
            mod._hook = h

        mod.set_axon_ntff_profile_hook = _set
        sys.modules["antenv.axon_hooks"] = mod
        return True
    except Exception:
        return False


def kernel(**inputs):
    global last_exec_time_ns
    from concourse.bass_utils import run_bass_kernel_spmd

    nc = _get_nc()
    in_maps = _pack_core_inputs(**inputs)

    trace = os.environ.get("KERNEL_PROFILE", "") == "1" and _install_ntff_hook()
    res = run_bass_kernel_spmd(
        nc, in_maps, list(range(N_CORES)), trace=trace
    )
    last_exec_time_ns = res.exec_time_ns

    S = np.zeros(6, dtype=np.float64)
    for r in res.results:
        S += r["partials"].reshape(6).astype(np.float64)

    cls = (S[0] + S[1]) / (N_POS + N_NEG)
    reg_v = (S[2] + S[3]) / (N_POS * 2)
    cnt = S[5]
    reg_o = (S[4] / max(cnt, 1.0)) if cnt > 0 else 0.0
    loss = cls + 1.0 * reg_v + 2.0 * reg_o
    return (
        np.float32(loss),
        np.float32(cls),
        np.float32(reg_v),
        np.float32(reg_o),
    )
